# revision 35
# baseline (speedup 1.0000x reference)
"""nn_Attention_18700287607351 — GQA attention (RMSNorm + RoPE, causal) on 8 trn2 cores.

Sharding: 8 shards = (batch b in {0,1}) x (4 KV-head groups per batch); each
shard owns 2 KV heads + their 8 query heads (GQA repeat local). Wq/Wk/Wv rows
and Wo columns split by head group; per-shard partial Wo products for a batch
are summed on the host (row-parallel output projection).

Device kernel (identical SPMD program, per-core weight slices as inputs):
  - projections as bf16 matmuls producing qT/kT [hd, t] directly (lhsT = W^T
    chunks, rhs = x^T chunks); v in natural [t, hd] via lhsT = x^T chunks
  - per-head RMS: squares (ACT) -> block-ones matmul -> exp(-0.5*ln(ms+eps))
    (Ln/Exp share one ACT table set with softmax exp) -> gpsimd partition
    broadcast -> DVE multiply
  - RoPE with host-deinterleaved head dims: tables carry cos/sin with the
    norm weights folded in; partition swap via SBUF-SBUF DMA; 2 mul + 1 add
  - scores^T [s, t] = kT.T @ qT with K=64 matmuls packed in pairs on PE row
    groups; softmax without max-subtraction (|logit| <= 8 after RMS) as a
    single ACT exp per tile pair; causal = skip upper blocks + one 0/1 wedge
    multiply per diagonal block (gpsimd)
  - attn @ v with a ones-column in v producing the softmax denominator row;
    normalize with DVE reciprocal + gpsimd broadcast
  - output projection from oT [f, t] tiles against Wo^T chunks; fp32 result
"""

import os
import numpy as np
import ml_dtypes

B, T, D = 2, 2048, 2048
H, HKV, HD = 32, 8, 64
THETA = 3.0
EPS = 1e-6
N_CORES = 8
GROUPS_PER_B = 4          # head-group shards per batch
KV_PER_G = 2              # KV heads per shard
QH_PER_G = 8              # query heads per shard
NP_ = 128                 # partitions
NT = 4                    # t blocks
TB = 512                  # t block size
NC_CHUNKS = D // NP_      # 16 contraction chunks
BF16 = ml_dtypes.bfloat16

_PROG = None  # cached (nc, input_names)


def build_program(nc=None):
    """Build the single-core Bass/Tile program (same for all 8 cores)."""
    import concourse.bass as bass
    import concourse.mybir as mybir
    import concourse.tile as tile
    from concourse import bacc

    f32 = mybir.dt.float32
    bf16 = mybir.dt.bfloat16
    AF = mybir.ActivationFunctionType

    class _Bacc(bacc.Bacc):
        def insert_act_table_loads(self):
            # All ACT funcs used here (Exp/Ln/Copy/Identity) live in ONE
            # table set; the default first-match choice alternates between
            # exp_and_others and natural_log, costing a ~1.5us table DMA
            # per transition. Empty out every other set (indices must be
            # preserved -- act_func_set_id is the list position).
            import bass_rust
            from concourse.hw_specs import get_activation_tables
            keep = "natural_log_exp_and_others"
            tables = [
                (n, s if n == keep else set())
                for n, s in get_activation_tables(self.m.arch).items()
            ]
            bass_rust.insert_act_table_loads(self, tables)

    if nc is None:
        nc = _Bacc(None, target_bir_lowering=False)

    xT_d = nc.dram_tensor("xT", [D, T], bf16, kind="ExternalInput")
    wq_d = nc.dram_tensor("wq", [D, 512], bf16, kind="ExternalInput")
    wk_d = nc.dram_tensor("wk", [D, 128], bf16, kind="ExternalInput")
    wv_d = nc.dram_tensor("wv", [D, 128], bf16, kind="ExternalInput")
    wo_d = nc.dram_tensor("wo", [512, D], bf16, kind="ExternalInput")
    cq_d = nc.dram_tensor("cq", [NP_, T], bf16, kind="ExternalInput")
    sq_d = nc.dram_tensor("sq", [NP_, T], bf16, kind="ExternalInput")
    ck_d = nc.dram_tensor("ck", [NP_, T], bf16, kind="ExternalInput")
    sk_d = nc.dram_tensor("sk", [NP_, T], bf16, kind="ExternalInput")
    tri_d = nc.dram_tensor("tri", [NP_, NP_], bf16, kind="ExternalInput")
    bw_d = nc.dram_tensor("bw", [NP_, NP_], bf16, kind="ExternalInput")
    y_d = nc.dram_tensor("y", [T, D], bf16, kind="ExternalOutput")

    with tile.TileContext(nc) as tc:
        with (
            tc.tile_pool(name="singles", bufs=1) as singles,
            tc.tile_pool(name="xpool", bufs=2) as xpool,
            tc.tile_pool(name="tmp", bufs=2) as tmp,
            tc.tile_pool(name="ppool", bufs=4) as ppool,
            tc.tile_pool(name="small", bufs=2) as small,
            tc.tile_pool(name="ypool", bufs=4) as ypool,
            tc.tile_pool(name="pwork", bufs=2, space="PSUM") as pwork,
            tc.tile_pool(name="po", bufs=2, space="PSUM") as po,
            tc.tile_pool(name="psc", bufs=2, space="PSUM") as psc,
        ):
            # ---- resident tensors -------------------------------------
            # ordered/spread so the first projection matmuls aren't stuck
            # behind the phase-C weights on one DMA queue
            wq_sb = singles.tile([NP_, NC_CHUNKS, 512], bf16)
            for g in range(4):
                nc.scalar.dma_start(
                    out=wq_sb[:, 4 * g:4 * g + 4, :],
                    in_=wq_d.rearrange("(c p) f -> p c f", p=NP_)[:, 4 * g:4 * g + 4, :])
            wk_sb = singles.tile([NP_, NC_CHUNKS, 128], bf16)
            wv_sb = singles.tile([NP_, NC_CHUNKS, 128], bf16)
            # tables/wo are needed only after the first projection matmuls;
            # queue them on sync BEHIND the first x block so the critical
            # ~5MB isn't sharing HBM bandwidth with them at kernel start
            bw_sb = singles.tile([NP_, NP_], bf16)
            nc.scalar.dma_start(out=bw_sb[:], in_=bw_d[:, :])
            cq_sb = singles.tile([NP_, T], bf16)
            sq_sb = singles.tile([NP_, T], bf16)
            ck_sb = singles.tile([NP_, T], bf16)
            sk_sb = singles.tile([NP_, T], bf16)
            tri_sb = singles.tile([NP_, NP_], bf16)
            wo_sb = singles.tile([NP_, 4, D], bf16)
            eps_sb = singles.tile([NP_, 1], f32)
            nc.vector.memset(eps_sb[:], EPS)
            deferred_loads = [False]

            def load_deferred():
                deferred_loads[0] = True
                nc.sync.dma_start(out=wk_sb[:], in_=wk_d.rearrange("(c p) f -> p c f", p=NP_))
                nc.sync.dma_start(out=wv_sb[:], in_=wv_d.rearrange("(c p) f -> p c f", p=NP_))
                nc.sync.dma_start(out=cq_sb[:], in_=cq_d[:, :])
                nc.sync.dma_start(out=sq_sb[:], in_=sq_d[:, :])
                nc.sync.dma_start(out=ck_sb[:], in_=ck_d[:, :])
                nc.sync.dma_start(out=sk_sb[:], in_=sk_d[:, :])
                nc.sync.dma_start(out=tri_sb[:], in_=tri_d[:, :])
                nc.gpsimd.dma_start(
                    out=wo_sb[:], in_=wo_d.rearrange("(c p) f -> p c f", p=NP_))

            qt_sb = [singles.tile([NP_, T], bf16, name=f"qt{p}", tag=f"qt{p}") for p in range(4)]
            kt_sb = singles.tile([NP_, T], bf16, name="kt_all", tag="kt_all")
            v_sb = [singles.tile([NP_, 16, 65], bf16, name=f"v{k}", tag=f"v{k}") for k in range(2)]
            oT_sb = [singles.tile([NP_, T], bf16, name=f"oT{p}", tag=f"oT{p}") for p in range(4)]
            # unnormalized attention outputs, one [64, T] tile per head so
            # every normalize multiply has both SBUF inputs at partition 0
            oTr_sb = [singles.tile([64, T], bf16, name=f"oTr{h}", tag=f"oTr{h}")
                      for h in range(8)]
            for k in range(2):
                nc.vector.memset(v_sb[k][:, :, 64:65], 1.0)

            # ---- helper: RMS + RoPE on a projected pair tile ----------
            # split in two so the next pair's projection matmuls can be
            # emitted between the PSUM-freeing copy and the RMS tail (keeps
            # the two work slots rotating between projection chains only)
            def rms_start(ps_tile):
                qf = tmp.tile([NP_, TB], f32, tag="qf", bufs=3)
                nc.vector.tensor_copy(qf[:], ps_tile[:])
                sqv = tmp.tile([NP_, TB], bf16, tag="sqv", bufs=3)
                nc.vector.tensor_mul(sqv[:], qf[:], qf[:])
                return qf, sqv

            def rms_finish(qf, sqv, tcols, cos_sb, sin_sb, out_ap):
                # per-head sum over the 64 hd partitions via block-ones
                # matmul; output is already in broadcast form [128, TB]
                ssb = pwork.tile([NP_, TB], f32, tag="work")
                nc.tensor.matmul(ssb[:], bw_sb[:], sqv[:], start=True, stop=True)
                # rsqrt(mean + eps) = exp(-0.5 * ln(ss/HD + eps))
                t1 = small.tile([NP_, TB], f32, tag="t1")
                nc.scalar.activation(t1[:], ssb[:], AF.Ln, scale=1.0 / HD,
                                     bias=eps_sb[:, :])
                rsb = small.tile([NP_, TB], f32, tag="rsb")
                nc.scalar.activation(rsb[:], t1[:], AF.Exp, scale=-0.5)
                qn = tmp.tile([NP_, TB], bf16, tag="qn")
                nc.vector.tensor_mul(qn[:], qf[:], rsb[:])
                # partition swap halves within each 64-row head
                qs = tmp.tile([NP_, TB], bf16, tag="qs")
                nc.sync.dma_start(out=qs[0:32, :], in_=qn[32:64, :])
                nc.gpsimd.dma_start(out=qs[32:64, :], in_=qn[0:32, :])
                nc.sync.dma_start(out=qs[64:96, :], in_=qn[96:NP_, :])
                nc.gpsimd.dma_start(out=qs[96:NP_, :], in_=qn[64:96, :])
                ta = tmp.tile([NP_, TB], bf16, tag="ta")
                nc.vector.tensor_mul(ta[:], qn[:], cos_sb[:, tcols])
                tb2 = tmp.tile([NP_, TB], bf16, tag="tb2")
                nc.vector.tensor_mul(tb2[:], qs[:], sin_sb[:, tcols])
                nc.vector.tensor_add(out_ap, ta[:], tb2[:])

            # ---- attention block for one (pair, t-block) --------------
            def attn_block(pair, tb):
                    ns = 4 * tb + 4
                    oA = po.tile([NP_, TB], f32, tag="o")
                    oB = po.tile([NP_, TB], f32, tag="o")
                    for j in range(ns):
                        diag_k = j - 4 * tb
                        c0 = 0 if diag_k < 0 else 128 * diag_k
                        vw = TB - c0
                        tc0 = tb * TB + c0
                        sc = psc.tile([NP_, 2, TB], f32, tag="sc")
                        srows = bass.ts(j, 128)
                        nc.tensor.matmul(
                            sc[:, 0, c0:TB],
                            kt_sb[0:64, srows],
                            qt_sb[pair][0:64, tc0:tb * TB + TB],
                            start=True, stop=True, tile_position=(0, 0),
                        )
                        nc.tensor.matmul(
                            sc[:, 1, c0:TB],
                            kt_sb[64:NP_, srows],
                            qt_sb[pair][64:NP_, tc0:tb * TB + TB],
                            start=True, stop=True, tile_position=(64, 0),
                        )
                        p_t = ppool.tile([NP_, 2, TB], bf16, tag="p")
                        nc.scalar.activation(
                            p_t[:, :, c0:TB], sc[:, :, c0:TB], AF.Exp,
                            scale=float(HD) ** -0.5,
                        )
                        if diag_k >= 0:
                            nc.vector.tensor_mul(
                                p_t[:, :, c0:c0 + 128],
                                p_t[:, :, c0:c0 + 128],
                                tri_sb[:, None, :].to_broadcast([NP_, 2, 128]),
                            )
                        nc.tensor.matmul(
                            oA[0:65, c0:TB], v_sb[0][:, j, :], p_t[:, 0, c0:TB],
                            start=(j == 0), stop=(j == ns - 1),
                            skip_group_check=True,
                        )
                        nc.tensor.matmul(
                            oB[0:65, c0:TB], v_sb[1][:, j, :], p_t[:, 1, c0:TB],
                            start=(j == 0), stop=(j == ns - 1),
                            skip_group_check=True,
                        )
                    for h01, oPs in ((0, oA), (1, oB)):
                        # drain the PSUM accumulator fast (copy + Ln of the
                        # denominator row), then finish the normalize chain
                        # from SBUF off the o-slot critical path.
                        nc.vector.tensor_copy(
                            oTr_sb[2 * pair + h01][:, bass.ts(tb, TB)],
                            oPs[0:64, :])
                        lnl = small.tile([1, TB], f32, tag="lnl")
                        nc.scalar.activation(lnl[:], oPs[64:65, :], AF.Ln)
                        # 1/l via exp(-ln(l)) -- same ACT table set as the
                        # softmax exp; DVE reciprocal is ~3.3us on [1, TB]
                        rcp = small.tile([1, TB], bf16, tag="rcp")
                        nc.scalar.activation(rcp[:], lnl[:], AF.Exp, scale=-1.0)
                        rb = small.tile([64, TB], bf16, tag="rb")
                        nc.gpsimd.partition_broadcast(rb[:], rcp[:])
                        nc.vector.tensor_mul(
                            oT_sb[pair][bass.ts(h01, 64), bass.ts(tb, TB)],
                            oTr_sb[2 * pair + h01][:, bass.ts(tb, TB)], rb[:],
                        )

            # ---- phases A+B interleaved (flash order) -----------------
            for tb in range(NT):
                tcols = bass.ts(tb, TB)
                xt = xpool.tile([NP_, NC_CHUNKS, TB], bf16, tag="xt")
                for g in range(4):
                    nc.sync.dma_start(
                        out=xt[:, 4 * g:4 * g + 4, :],
                        in_=xT_d.rearrange("(c p) t -> p c t", p=NP_)[
                            :, 4 * g:4 * g + 4, tcols],
                    )
                if not deferred_loads[0]:
                    load_deferred()
                pending = None
                for pair in range(4):
                    qps = pwork.tile([NP_, TB], f32, tag="work")
                    for c in range(NC_CHUNKS):
                        nc.tensor.matmul(
                            qps[:], wq_sb[:, c, bass.ts(pair, 128)], xt[:, c, :],
                            start=(c == 0), stop=(c == NC_CHUNKS - 1),
                        )
                    st = rms_start(qps)
                    if pending is not None:
                        rms_finish(*pending)
                    pending = (*st, tcols, cq_sb, sq_sb, qt_sb[pair][:, tcols])
                # k (one pair of KV heads)
                kps = pwork.tile([NP_, TB], f32, tag="work")
                for c in range(NC_CHUNKS):
                    nc.tensor.matmul(
                        kps[:], wk_sb[:, c, :], xt[:, c, :],
                        start=(c == 0), stop=(c == NC_CHUNKS - 1),
                    )
                st = rms_start(kps)
                rms_finish(*pending)
                rms_finish(*st, tcols, ck_sb, sk_sb, kt_sb[:, tcols])
                # v for the 4 s-tiles of this block
                for st in range(4):
                    j = 4 * tb + st
                    vps = pwork.tile([NP_, 128], f32, tag="work")
                    for c in range(NC_CHUNKS):
                        nc.tensor.matmul(
                            vps[:], xt[:, c, bass.ts(st, 128)], wv_sb[:, c, :],
                            start=(c == 0), stop=(c == NC_CHUNKS - 1),
                        )
                    nc.vector.tensor_copy(v_sb[0][:, j, 0:64], vps[:, 0:64])
                    nc.vector.tensor_copy(v_sb[1][:, j, 0:64], vps[:, 64:128])
                # attention over everything this block completed
                for pair in range(4):
                    attn_block(pair, tb)
                # output projection for the t-rows this block finalized
                for tt in range(4 * tb, 4 * tb + 4):
                    trows = bass.ts(tt, 128)
                    for db in range(4):
                        if db % 2 == 0:
                            yps = pwork.tile([NP_, TB], f32, tag="work")
                        else:
                            yps = po.tile([NP_, TB], f32, tag="o")
                        for pair in range(4):
                            nc.tensor.matmul(
                                yps[:], oT_sb[pair][:, trows],
                                wo_sb[:, pair, bass.ts(db, TB)],
                                start=(pair == 0), stop=(pair == 3),
                            )
                        ycp = ypool.tile([NP_, TB], bf16, tag="ycp")
                        nc.vector.tensor_copy(ycp[:], yps[:])
                        nc.sync.dma_start(out=y_d[trows, bass.ts(db, TB)], in_=ycp[:])

    return nc


def _rope_tables(norm_w):
    """cos/sin tables [128, T] with the RMS norm weight folded in."""
    j = np.arange(32)
    inv_freq = 1.0 / (THETA ** (np.arange(0, HD, 2, dtype=np.float64) / HD))
    t = np.arange(T, dtype=np.float64)
    f = t[None, :] * inv_freq[:, None]          # [32, T]
    c, s = np.cos(f), np.sin(f)
    w = np.asarray(norm_w, dtype=np.float64)
    w_e, w_o = w[0::2][:, None], w[1::2][:, None]
    cq = np.empty((NP_, T), dtype=np.float64)
    sq = np.empty((NP_, T), dtype=np.float64)
    cq[0:32] = w_e * c
    cq[32:64] = w_o * c
    sq[0:32] = -w_o * s
    sq[32:64] = w_e * s
    cq[64:96], cq[96:128] = cq[0:32], cq[32:64]
    sq[64:96], sq[96:128] = sq[0:32], sq[32:64]
    return cq.astype(BF16), sq.astype(BF16)


def _prep_inputs(x, Wq, Wk, Wv, Wo, q_norm_w, k_norm_w):
    """Per-core input maps (host-side sharding, layout, casts)."""
    perm = np.empty(HD, dtype=np.int64)
    perm[0:32] = np.arange(32) * 2
    perm[32:64] = np.arange(32) * 2 + 1

    cq, sq = _rope_tables(q_norm_w)
    ck, sk = _rope_tables(k_norm_w)
    tri = (np.arange(NP_)[None, :] >= np.arange(NP_)[:, None]).astype(BF16)
    half = np.arange(NP_) < 64
    bw = (half[:, None] == half[None, :]).astype(BF16)

    xT = [np.ascontiguousarray(x[b].T).astype(BF16) for b in range(B)]

    in_maps = []
    for core in range(N_CORES):
        b, g = divmod(core, GROUPS_PER_B)
        qh0 = g * QH_PER_G
        kv0 = g * KV_PER_G
        head_order = [0, 4, 1, 5, 2, 6, 3, 7]
        wq = np.empty((D, 512), dtype=np.float32)
        for slot, h in enumerate(head_order):
            wq[:, 64 * slot:64 * slot + 64] = Wq[(qh0 + h) * HD + perm, :].T
        wk = np.empty((D, 128), dtype=np.float32)
        wv = np.empty((D, 128), dtype=np.float32)
        for k in range(KV_PER_G):
            wk[:, 64 * k:64 * k + 64] = Wk[(kv0 + k) * HD + perm, :].T
            wv[:, 64 * k:64 * k + 64] = Wv[(kv0 + k) * HD:(kv0 + k + 1) * HD, :].T
        wo = np.empty((512, D), dtype=np.float32)
        for slot, h in enumerate(head_order):
            wo[64 * slot:64 * slot + 64, :] = Wo[:, (qh0 + h) * HD:(qh0 + h + 1) * HD].T
        in_maps.append({
            "xT": xT[b],
            "wq": wq.astype(BF16), "wk": wk.astype(BF16),
            "wv": wv.astype(BF16), "wo": wo.astype(BF16),
            "cq": cq, "sq": sq, "ck": ck, "sk": sk,
            "tri": tri, "bw": bw,
        })
    return in_maps


def _get_prog():
    global _PROG
    if _PROG is None:
        nc = build_program()
        nc.finalize()
        _PROG = nc
    return _PROG


def kernel(x, Wq, Wk, Wv, Wo, q_norm_w, k_norm_w, mask=None, **_unused):
    from concourse.bass_utils import run_bass_kernel_spmd

    x = np.asarray(x, dtype=np.float32)
    in_maps = _prep_inputs(
        x,
        np.asarray(Wq, dtype=np.float32), np.asarray(Wk, dtype=np.float32),
        np.asarray(Wv, dtype=np.float32), np.asarray(Wo, dtype=np.float32),
        np.asarray(q_norm_w, dtype=np.float32),
        np.asarray(k_norm_w, dtype=np.float32),
    )
    nc = _get_prog()
    res = run_bass_kernel_spmd(nc, in_maps, list(range(N_CORES)))
    out = np.zeros((B, T, D), dtype=np.float32)
    for core in range(N_CORES):
        out[core // GROUPS_PER_B] += res.results[core]["y"].astype(np.float32)
    return out


# revision 36
# speedup vs baseline: 1.1821x; 1.1821x over previous
"""nn_Attention_18700287607351 — GQA attention (RMSNorm + RoPE, causal) on 8 trn2 cores.

Sharding: 8 shards = (batch b in {0,1}) x (4 KV-head groups per batch); each
shard owns 2 KV heads + their 8 query heads (GQA repeat local). Wq/Wk/Wv rows
and Wo columns split by head group; per-shard partial Wo products for a batch
are summed on the host (row-parallel output projection).

Device kernel (identical SPMD program, per-core weight slices as inputs):
  - projections as bf16 matmuls producing qT/kT [hd, t] directly (lhsT = W^T
    chunks, rhs = x^T chunks); v in natural [t, hd] via lhsT = x^T chunks
  - per-head RMS: squares (ACT) -> block-ones matmul -> exp(-0.5*ln(ms+eps))
    (Ln/Exp share one ACT table set with softmax exp) -> gpsimd partition
    broadcast -> DVE multiply
  - RoPE with host-deinterleaved head dims: tables carry cos/sin with the
    norm weights folded in; partition swap via SBUF-SBUF DMA; 2 mul + 1 add
  - scores^T [s, t] = kT.T @ qT with K=64 matmuls packed in pairs on PE row
    groups; softmax without max-subtraction (|logit| <= 8 after RMS) as a
    single ACT exp per tile pair; causal = skip upper blocks + one 0/1 wedge
    multiply per diagonal block (gpsimd)
  - attn @ v with a ones-column in v producing the softmax denominator row;
    normalize with DVE reciprocal + gpsimd broadcast
  - output projection from oT [f, t] tiles against Wo^T chunks; fp32 result
"""

import os
import numpy as np
import ml_dtypes

B, T, D = 2, 2048, 2048
H, HKV, HD = 32, 8, 64
THETA = 3.0
EPS = 1e-6
N_CORES = 8
GROUPS_PER_B = 4          # head-group shards per batch
KV_PER_G = 2              # KV heads per shard
QH_PER_G = 8              # query heads per shard
NP_ = 128                 # partitions
NT = 4                    # t blocks
TB = 512                  # t block size
NC_CHUNKS = D // NP_      # 16 contraction chunks
BF16 = ml_dtypes.bfloat16

_PROG = None  # cached (nc, input_names)


def build_program(nc=None):
    """Build the single-core Bass/Tile program (same for all 8 cores)."""
    import concourse.bass as bass
    import concourse.mybir as mybir
    import concourse.tile as tile
    from concourse import bacc

    f32 = mybir.dt.float32
    bf16 = mybir.dt.bfloat16
    AF = mybir.ActivationFunctionType

    class _Bacc(bacc.Bacc):
        def insert_act_table_loads(self):
            # All ACT funcs used here (Exp/Ln/Copy/Identity) live in ONE
            # table set; the default first-match choice alternates between
            # exp_and_others and natural_log, costing a ~1.5us table DMA
            # per transition. Empty out every other set (indices must be
            # preserved -- act_func_set_id is the list position).
            import bass_rust
            from concourse.hw_specs import get_activation_tables
            keep = "natural_log_exp_and_others"
            tables = [
                (n, s if n == keep else set())
                for n, s in get_activation_tables(self.m.arch).items()
            ]
            bass_rust.insert_act_table_loads(self, tables)

    if nc is None:
        nc = _Bacc(None, target_bir_lowering=False)

    xT_d = nc.dram_tensor("xT", [D, T], bf16, kind="ExternalInput")
    wq_d = nc.dram_tensor("wq", [D, 512], bf16, kind="ExternalInput")
    wk_d = nc.dram_tensor("wk", [D, 128], bf16, kind="ExternalInput")
    wv_d = nc.dram_tensor("wv", [D, 128], bf16, kind="ExternalInput")
    wo_d = nc.dram_tensor("wo", [512, D], bf16, kind="ExternalInput")
    cq_d = nc.dram_tensor("cq", [NP_, T], bf16, kind="ExternalInput")
    sq_d = nc.dram_tensor("sq", [NP_, T], bf16, kind="ExternalInput")
    ck_d = nc.dram_tensor("ck", [NP_, T], bf16, kind="ExternalInput")
    sk_d = nc.dram_tensor("sk", [NP_, T], bf16, kind="ExternalInput")
    tri_d = nc.dram_tensor("tri", [NP_, NP_], bf16, kind="ExternalInput")
    bw_d = nc.dram_tensor("bw", [NP_, NP_], bf16, kind="ExternalInput")
    y_d = nc.dram_tensor("y", [T, D], bf16, kind="ExternalOutput")

    with tile.TileContext(nc) as tc:
        with (
            tc.tile_pool(name="singles", bufs=1) as singles,
            tc.tile_pool(name="xpool", bufs=2) as xpool,
            tc.tile_pool(name="tmp", bufs=2) as tmp,
            tc.tile_pool(name="ppool", bufs=4) as ppool,
            tc.tile_pool(name="small", bufs=2) as small,
            tc.tile_pool(name="ypool", bufs=4) as ypool,
            tc.tile_pool(name="pwork", bufs=2, space="PSUM") as pwork,
            tc.tile_pool(name="po", bufs=2, space="PSUM") as po,
            tc.tile_pool(name="psc", bufs=2, space="PSUM") as psc,
        ):
            # ---- resident tensors -------------------------------------
            # ordered/spread so the first projection matmuls aren't stuck
            # behind the phase-C weights on one DMA queue
            wq_sb = singles.tile([NP_, NC_CHUNKS, 512], bf16)
            for g in range(4):
                nc.scalar.dma_start(
                    out=wq_sb[:, 4 * g:4 * g + 4, :],
                    in_=wq_d.rearrange("(c p) f -> p c f", p=NP_)[:, 4 * g:4 * g + 4, :])
            wk_sb = singles.tile([NP_, NC_CHUNKS, 128], bf16)
            nc.sync.dma_start(out=wk_sb[:], in_=wk_d.rearrange("(c p) f -> p c f", p=NP_))
            wv_sb = singles.tile([NP_, NC_CHUNKS, 128], bf16)
            nc.sync.dma_start(out=wv_sb[:], in_=wv_d.rearrange("(c p) f -> p c f", p=NP_))
            # tables/wo are needed only after the first projection matmuls;
            # queue them on sync BEHIND the first x block so the critical
            # ~5MB isn't sharing HBM bandwidth with them at kernel start
            bw_sb = singles.tile([NP_, NP_], bf16)
            nc.scalar.dma_start(out=bw_sb[:], in_=bw_d[:, :])
            cq_sb = singles.tile([NP_, T], bf16)
            sq_sb = singles.tile([NP_, T], bf16)
            ck_sb = singles.tile([NP_, T], bf16)
            sk_sb = singles.tile([NP_, T], bf16)
            tri_sb = singles.tile([NP_, NP_], bf16)
            wo_sb = singles.tile([NP_, 4, D], bf16)
            eps_sb = singles.tile([NP_, 1], f32)
            nc.vector.memset(eps_sb[:], EPS)
            deferred_loads = [False]

            def load_deferred():
                deferred_loads[0] = True
                nc.sync.dma_start(out=cq_sb[:], in_=cq_d[:, :])
                nc.sync.dma_start(out=sq_sb[:], in_=sq_d[:, :])
                nc.sync.dma_start(out=ck_sb[:], in_=ck_d[:, :])
                nc.sync.dma_start(out=sk_sb[:], in_=sk_d[:, :])
                nc.sync.dma_start(out=tri_sb[:], in_=tri_d[:, :])
                nc.gpsimd.dma_start(
                    out=wo_sb[:], in_=wo_d.rearrange("(c p) f -> p c f", p=NP_))

            qt_sb = [singles.tile([NP_, T], bf16, name=f"qt{p}", tag=f"qt{p}") for p in range(4)]
            kt_sb = singles.tile([NP_, T], bf16, name="kt_all", tag="kt_all")
            v_sb = [singles.tile([NP_, 16, 65], bf16, name=f"v{k}", tag=f"v{k}") for k in range(2)]
            oT_sb = [singles.tile([NP_, T], bf16, name=f"oT{p}", tag=f"oT{p}") for p in range(4)]
            # unnormalized attention outputs, one [64, T] tile per head so
            # every normalize multiply has both SBUF inputs at partition 0
            oTr_sb = [singles.tile([64, T], bf16, name=f"oTr{h}", tag=f"oTr{h}")
                      for h in range(8)]
            for k in range(2):
                nc.vector.memset(v_sb[k][:, :, 64:65], 1.0)

            # ---- helper: RMS + RoPE on a projected pair tile ----------
            # split in two so the next pair's projection matmuls can be
            # emitted between the PSUM-freeing copy and the RMS tail (keeps
            # the two work slots rotating between projection chains only)
            def rms_start(ps_tile):
                qf = tmp.tile([NP_, TB], f32, tag="qf", bufs=3)
                nc.vector.tensor_copy(qf[:], ps_tile[:])
                sqv = tmp.tile([NP_, TB], bf16, tag="sqv", bufs=3)
                nc.vector.tensor_mul(sqv[:], qf[:], qf[:])
                return qf, sqv

            def rms_finish(qf, sqv, tcols, cos_sb, sin_sb, out_ap):
                # per-head sum over the 64 hd partitions via block-ones
                # matmul; output is already in broadcast form [128, TB]
                ssb = pwork.tile([NP_, TB], f32, tag="work")
                nc.tensor.matmul(ssb[:], bw_sb[:], sqv[:], start=True, stop=True)
                # rsqrt(mean + eps) = exp(-0.5 * ln(ss/HD + eps))
                t1 = small.tile([NP_, TB], f32, tag="t1")
                nc.scalar.activation(t1[:], ssb[:], AF.Ln, scale=1.0 / HD,
                                     bias=eps_sb[:, :])
                rsb = small.tile([NP_, TB], f32, tag="rsb")
                nc.scalar.activation(rsb[:], t1[:], AF.Exp, scale=-0.5)
                qn = tmp.tile([NP_, TB], bf16, tag="qn")
                nc.vector.tensor_mul(qn[:], qf[:], rsb[:])
                # partition swap halves within each 64-row head
                qs = tmp.tile([NP_, TB], bf16, tag="qs")
                nc.sync.dma_start(out=qs[0:32, :], in_=qn[32:64, :])
                nc.gpsimd.dma_start(out=qs[32:64, :], in_=qn[0:32, :])
                nc.sync.dma_start(out=qs[64:96, :], in_=qn[96:NP_, :])
                nc.gpsimd.dma_start(out=qs[96:NP_, :], in_=qn[64:96, :])
                ta = tmp.tile([NP_, TB], bf16, tag="ta")
                nc.vector.tensor_mul(ta[:], qn[:], cos_sb[:, tcols])
                tb2 = tmp.tile([NP_, TB], bf16, tag="tb2")
                nc.vector.tensor_mul(tb2[:], qs[:], sin_sb[:, tcols])
                nc.vector.tensor_add(out_ap, ta[:], tb2[:])

            # ---- attention block for one (pair, t-block) --------------
            def attn_block(pair, tb):
                    ns = 4 * tb + 4
                    oA = po.tile([NP_, TB], f32, tag="o")
                    oB = po.tile([NP_, TB], f32, tag="o")
                    for j in range(ns):
                        diag_k = j - 4 * tb
                        c0 = 0 if diag_k < 0 else 128 * diag_k
                        vw = TB - c0
                        tc0 = tb * TB + c0
                        sc = psc.tile([NP_, 2, TB], f32, tag="sc")
                        srows = bass.ts(j, 128)
                        nc.tensor.matmul(
                            sc[:, 0, c0:TB],
                            kt_sb[0:64, srows],
                            qt_sb[pair][0:64, tc0:tb * TB + TB],
                            start=True, stop=True, tile_position=(0, 0),
                        )
                        nc.tensor.matmul(
                            sc[:, 1, c0:TB],
                            kt_sb[64:NP_, srows],
                            qt_sb[pair][64:NP_, tc0:tb * TB + TB],
                            start=True, stop=True, tile_position=(64, 0),
                        )
                        p_t = ppool.tile([NP_, 2, TB], bf16, tag="p")
                        nc.scalar.activation(
                            p_t[:, :, c0:TB], sc[:, :, c0:TB], AF.Exp,
                            scale=float(HD) ** -0.5,
                        )
                        if diag_k >= 0:
                            nc.vector.tensor_mul(
                                p_t[:, :, c0:c0 + 128],
                                p_t[:, :, c0:c0 + 128],
                                tri_sb[:, None, :].to_broadcast([NP_, 2, 128]),
                            )
                        nc.tensor.matmul(
                            oA[0:65, c0:TB], v_sb[0][:, j, :], p_t[:, 0, c0:TB],
                            start=(j == 0), stop=(j == ns - 1),
                            skip_group_check=True,
                        )
                        nc.tensor.matmul(
                            oB[0:65, c0:TB], v_sb[1][:, j, :], p_t[:, 1, c0:TB],
                            start=(j == 0), stop=(j == ns - 1),
                            skip_group_check=True,
                        )
                    for h01, oPs in ((0, oA), (1, oB)):
                        # drain the PSUM accumulator fast (copy + Ln of the
                        # denominator row), then finish the normalize chain
                        # from SBUF off the o-slot critical path.
                        nc.vector.tensor_copy(
                            oTr_sb[2 * pair + h01][:, bass.ts(tb, TB)],
                            oPs[0:64, :])
                        lnl = small.tile([1, TB], f32, tag="lnl")
                        nc.scalar.activation(lnl[:], oPs[64:65, :], AF.Ln)
                        # 1/l via exp(-ln(l)) -- same ACT table set as the
                        # softmax exp; DVE reciprocal is ~3.3us on [1, TB]
                        rcp = small.tile([1, TB], bf16, tag="rcp")
                        nc.scalar.activation(rcp[:], lnl[:], AF.Exp, scale=-1.0)
                        rb = small.tile([64, TB], bf16, tag="rb")
                        nc.gpsimd.partition_broadcast(rb[:], rcp[:])
                        nc.vector.tensor_mul(
                            oT_sb[pair][bass.ts(h01, 64), bass.ts(tb, TB)],
                            oTr_sb[2 * pair + h01][:, bass.ts(tb, TB)], rb[:],
                        )

            # ---- phases A+B interleaved (flash order) -----------------
            for tb in range(NT):
                tcols = bass.ts(tb, TB)
                xt = xpool.tile([NP_, NC_CHUNKS, TB], bf16, tag="xt")
                for g in range(4):
                    nc.sync.dma_start(
                        out=xt[:, 4 * g:4 * g + 4, :],
                        in_=xT_d.rearrange("(c p) t -> p c t", p=NP_)[
                            :, 4 * g:4 * g + 4, tcols],
                    )
                if not deferred_loads[0]:
                    load_deferred()
                pending = None
                for pair in range(4):
                    qps = pwork.tile([NP_, TB], f32, tag="work")
                    for c in range(NC_CHUNKS):
                        nc.tensor.matmul(
                            qps[:], wq_sb[:, c, bass.ts(pair, 128)], xt[:, c, :],
                            start=(c == 0), stop=(c == NC_CHUNKS - 1),
                        )
                    st = rms_start(qps)
                    if pending is not None:
                        rms_finish(*pending)
                    pending = (*st, tcols, cq_sb, sq_sb, qt_sb[pair][:, tcols])
                # k (one pair of KV heads)
                kps = pwork.tile([NP_, TB], f32, tag="work")
                for c in range(NC_CHUNKS):
                    nc.tensor.matmul(
                        kps[:], wk_sb[:, c, :], xt[:, c, :],
                        start=(c == 0), stop=(c == NC_CHUNKS - 1),
                    )
                st = rms_start(kps)
                rms_finish(*pending)
                rms_finish(*st, tcols, ck_sb, sk_sb, kt_sb[:, tcols])
                # v for the 4 s-tiles of this block
                for st in range(4):
                    j = 4 * tb + st
                    vps = pwork.tile([NP_, 128], f32, tag="work")
                    for c in range(NC_CHUNKS):
                        nc.tensor.matmul(
                            vps[:], xt[:, c, bass.ts(st, 128)], wv_sb[:, c, :],
                            start=(c == 0), stop=(c == NC_CHUNKS - 1),
                        )
                    nc.vector.tensor_copy(v_sb[0][:, j, 0:64], vps[:, 0:64])
                    nc.vector.tensor_copy(v_sb[1][:, j, 0:64], vps[:, 64:128])
                # attention over everything this block completed
                for pair in range(4):
                    attn_block(pair, tb)
                # output projection for the t-rows this block finalized
                for tt in range(4 * tb, 4 * tb + 4):
                    trows = bass.ts(tt, 128)
                    for db in range(4):
                        if db % 2 == 0:
                            yps = pwork.tile([NP_, TB], f32, tag="work")
                        else:
                            yps = po.tile([NP_, TB], f32, tag="o")
                        for pair in range(4):
                            nc.tensor.matmul(
                                yps[:], oT_sb[pair][:, trows],
                                wo_sb[:, pair, bass.ts(db, TB)],
                                start=(pair == 0), stop=(pair == 3),
                            )
                        ycp = ypool.tile([NP_, TB], bf16, tag="ycp")
                        nc.vector.tensor_copy(ycp[:], yps[:])
                        nc.sync.dma_start(out=y_d[trows, bass.ts(db, TB)], in_=ycp[:])

    return nc


def _rope_tables(norm_w):
    """cos/sin tables [128, T] with the RMS norm weight folded in."""
    j = np.arange(32)
    inv_freq = 1.0 / (THETA ** (np.arange(0, HD, 2, dtype=np.float64) / HD))
    t = np.arange(T, dtype=np.float64)
    f = t[None, :] * inv_freq[:, None]          # [32, T]
    c, s = np.cos(f), np.sin(f)
    w = np.asarray(norm_w, dtype=np.float64)
    w_e, w_o = w[0::2][:, None], w[1::2][:, None]
    cq = np.empty((NP_, T), dtype=np.float64)
    sq = np.empty((NP_, T), dtype=np.float64)
    cq[0:32] = w_e * c
    cq[32:64] = w_o * c
    sq[0:32] = -w_o * s
    sq[32:64] = w_e * s
    cq[64:96], cq[96:128] = cq[0:32], cq[32:64]
    sq[64:96], sq[96:128] = sq[0:32], sq[32:64]
    return cq.astype(BF16), sq.astype(BF16)


def _prep_inputs(x, Wq, Wk, Wv, Wo, q_norm_w, k_norm_w):
    """Per-core input maps (host-side sharding, layout, casts)."""
    perm = np.empty(HD, dtype=np.int64)
    perm[0:32] = np.arange(32) * 2
    perm[32:64] = np.arange(32) * 2 + 1

    cq, sq = _rope_tables(q_norm_w)
    ck, sk = _rope_tables(k_norm_w)
    tri = (np.arange(NP_)[None, :] >= np.arange(NP_)[:, None]).astype(BF16)
    half = np.arange(NP_) < 64
    bw = (half[:, None] == half[None, :]).astype(BF16)

    xT = [np.ascontiguousarray(x[b].T).astype(BF16) for b in range(B)]

    in_maps = []
    for core in range(N_CORES):
        b, g = divmod(core, GROUPS_PER_B)
        qh0 = g * QH_PER_G
        kv0 = g * KV_PER_G
        head_order = [0, 4, 1, 5, 2, 6, 3, 7]
        wq = np.empty((D, 512), dtype=np.float32)
        for slot, h in enumerate(head_order):
            wq[:, 64 * slot:64 * slot + 64] = Wq[(qh0 + h) * HD + perm, :].T
        wk = np.empty((D, 128), dtype=np.float32)
        wv = np.empty((D, 128), dtype=np.float32)
        for k in range(KV_PER_G):
            wk[:, 64 * k:64 * k + 64] = Wk[(kv0 + k) * HD + perm, :].T
            wv[:, 64 * k:64 * k + 64] = Wv[(kv0 + k) * HD:(kv0 + k + 1) * HD, :].T
        wo = np.empty((512, D), dtype=np.float32)
        for slot, h in enumerate(head_order):
            wo[64 * slot:64 * slot + 64, :] = Wo[:, (qh0 + h) * HD:(qh0 + h + 1) * HD].T
        in_maps.append({
            "xT": xT[b],
            "wq": wq.astype(BF16), "wk": wk.astype(BF16),
            "wv": wv.astype(BF16), "wo": wo.astype(BF16),
            "cq": cq, "sq": sq, "ck": ck, "sk": sk,
            "tri": tri, "bw": bw,
        })
    return in_maps


def _get_prog():
    global _PROG
    if _PROG is None:
        nc = build_program()
        nc.finalize()
        _PROG = nc
    return _PROG


def kernel(x, Wq, Wk, Wv, Wo, q_norm_w, k_norm_w, mask=None, **_unused):
    from concourse.bass_utils import run_bass_kernel_spmd

    x = np.asarray(x, dtype=np.float32)
    in_maps = _prep_inputs(
        x,
        np.asarray(Wq, dtype=np.float32), np.asarray(Wk, dtype=np.float32),
        np.asarray(Wv, dtype=np.float32), np.asarray(Wo, dtype=np.float32),
        np.asarray(q_norm_w, dtype=np.float32),
        np.asarray(k_norm_w, dtype=np.float32),
    )
    nc = _get_prog()
    res = run_bass_kernel_spmd(nc, in_maps, list(range(N_CORES)))
    out = np.zeros((B, T, D), dtype=np.float32)
    for core in range(N_CORES):
        out[core // GROUPS_PER_B] += res.results[core]["y"].astype(np.float32)
    return out
